# revision 7
# baseline (speedup 1.0000x reference)
"""Trainium2 Bass kernel for nn_AgentTwo (ragged-sequence GRU agent).

Full-input contract: kernel(**inputs) takes the unsharded numpy inputs and
returns the full [8192, 10] float32 action probabilities.

Strategy (pure data parallel over 8 NeuronCores, B=8192 -> 1024 rows/core):
 - Host resolves the ragged aliveness up front: per row, tokens at steps at
   or after the first zero are rewritten to a sentinel whose (pre-negated)
   z-gate projection is -30000, so sigmoid saturates to 0 and h freezes
   exactly on device -- the reference's "output_state while alive" semantics
   fall out with zero extra device work.
 - Host resolves the embedding lookup AND the input-side projections: the
   per-core bf16 stream carries, per token, [i_r+b_r | -(i_z)-b_z | i_n+b_ihn]
   in [E, B] layout (E on partitions). The device then:
     * loads the projections into PSUM via an identity matmul and
       accumulates the hidden-side matmuls on top (PE, bf16 in / f32 acc),
     * applies BOTH sigmoids in a single unbiased ACTIVATE over the
       adjacent r/zneg PSUM banks,
     * runs the n-gate and the frozen-blend on DVE:
         tg   = (psum_hn + b_hhn) * r           (scalar_tensor_tensor)
         npre = tg + gi_n ; n = tanh(npre)      (DVE + ACT)
         h'   = h + zbar * (n - h)              (DVE x3, bf16)
   The batch runs as two independent 512-column streams so the two
   recurrence dependency chains pipeline across engines.
 - Head: logitsT = w_out @ h (PE), expv = exp(logitsT + b_out) (ACT); host
   normalizes the softmax in f64 and reassembles [8192, 10].
"""

import sys

for _p in ("/opt/trn_rl_repo",):
    if _p not in sys.path:
        sys.path.append(_p)

import numpy as np
import ml_dtypes

import concourse.bass as bass
import concourse.mybir as mybir
import concourse.tile as tile
from concourse import bacc
from concourse.bass_utils import run_bass_kernel_spmd

BF16 = ml_dtypes.bfloat16

NCORES = 8
B, T, E, V, A = 8192, 64, 128, 32000, 10
V1 = V + 1          # vocab rows (0..32000)
BL = B // NCORES    # 1024 rows per core
HALF = BL // 2      # 512-column stream width
GS = 2              # timesteps per stream DMA
NG = T // GS        # stream groups
F32 = mybir.dt.float32
BF = mybir.dt.bfloat16

_CACHE = {}


def _build_nc(T=T, BL=BL, NG=NG):
    HALF = BL // 2
    nc = bacc.Bacc(None)
    es_d = nc.declare_dram_parameter("estream", [NG, 128, GS * 3 * BL], BF, isOutput=False)
    w_d = nc.declare_dram_parameter("wstat", [128, 4 * E], BF, isOutput=False)
    bias_d = nc.declare_dram_parameter("biasp", [128, 1], F32, isOutput=False)
    wout_d = nc.declare_dram_parameter("woutT", [128, A], BF, isOutput=False)
    bout_d = nc.declare_dram_parameter("bout", [A, 1], F32, isOutput=False)
    out_d = nc.declare_dram_parameter("expv", [A, BL], F32, isOutput=True)

    SIG = mybir.ActivationFunctionType.Sigmoid
    TANH = mybir.ActivationFunctionType.Tanh
    EXP = mybir.ActivationFunctionType.Exp
    ADD = mybir.AluOpType.add
    MULT = mybir.AluOpType.mult

    with tile.TileContext(nc) as tc:
        with (
            tc.tile_pool(name="const", bufs=1) as cp,
            tc.tile_pool(name="gath", bufs=4) as gathp,
            tc.tile_pool(name="hA", bufs=3) as hpA,
            tc.tile_pool(name="hB", bufs=3) as hpB,
            tc.tile_pool(name="gates", bufs=3) as gp,
            tc.tile_pool(name="ps", bufs=1, space=bass.MemorySpace.PSUM) as psp,
        ):
            w_sb = cp.tile([128, 4 * E], BF, tag="w")
            bias_sb = cp.tile([128, 1], F32, tag="bias")
            wout_sb = cp.tile([128, A], BF, tag="wout")
            bout_sb = cp.tile([A, 1], F32, tag="bout")
            nc.sync.dma_start(w_sb[:], w_d[:])
            nc.sync.dma_start(bias_sb[:], bias_d[:])
            nc.sync.dma_start(wout_sb[:], wout_d[:])
            nc.sync.dma_start(bout_sb[:], bout_d[:])

            # weight column slices in w_sb: [I | hhr | -hhz | hhn]
            W_ID = w_sb[:, 0 * E:1 * E]
            W_HHR = w_sb[:, 1 * E:2 * E]
            W_HHZN = w_sb[:, 2 * E:3 * E]
            W_HHN = w_sb[:, 3 * E:4 * E]
            B_HHN = bias_sb[:, 0:1]

            h_cur = []
            for s, hp in ((0, hpA), (1, hpB)):
                h0 = hp.tile([128, HALF], BF, tag=f"h{s}")
                nc.vector.memset(h0[:], 0.0)
                h_cur.append(h0)

            for g in range(NG):
                ep = gathp.tile([128, GS, 3, BL], BF, tag="ep")
                nc.sync.dma_start(ep[:], es_d[g])
                for k in range(GS):
                    for s in range(2):
                        lo = s * HALF
                        hi = lo + HALF
                        prT = ep[:, k, 0, lo:hi]
                        pzT = ep[:, k, 1, lo:hi]
                        pnT = ep[:, k, 2, lo:hi]
                        h = h_cur[s]

                        ps_rz = psp.tile([128, 2 * HALF], F32, tag=f"rz{s}")
                        ps_hn = psp.tile([128, HALF], F32, tag=f"hn{s}")
                        # projection loads first: they only need the stream,
                        # so PE can run them while waiting for h'
                        nc.tensor.matmul(ps_rz[:, 0:HALF], W_ID, prT, start=True, stop=False)
                        nc.tensor.matmul(ps_rz[:, HALF:], W_ID, pzT, start=True, stop=False)
                        nc.tensor.matmul(ps_rz[:, 0:HALF], W_HHR, h[:], start=False, stop=True)
                        nc.tensor.matmul(ps_hn[:], W_HHN, h[:], start=True, stop=True)
                        nc.tensor.matmul(ps_rz[:, HALF:], W_HHZN, h[:], start=False, stop=True)

                        r = gp.tile([128, HALF], BF, tag=f"r{s}")
                        zb = gp.tile([128, HALF], BF, tag=f"zb{s}")
                        tg = gp.tile([128, HALF], BF, tag=f"tg{s}")
                        npre = gp.tile([128, HALF], BF, tag=f"np{s}")
                        n = gp.tile([128, HALF], BF, tag=f"n{s}")
                        hn2 = (hpA if s == 0 else hpB).tile([128, HALF], BF, tag=f"h{s}")

                        nc.scalar.activation(r[:], ps_rz[:, 0:HALF], SIG)
                        nc.vector.scalar_tensor_tensor(tg[:], ps_hn[:], B_HHN, r[:], ADD, MULT)
                        nc.vector.tensor_add(npre[:], tg[:], pnT)
                        nc.scalar.activation(zb[:], ps_rz[:, HALF:], SIG)
                        nc.scalar.activation(n[:], npre[:], TANH)
                        if s == 0:
                            # off-chain pair on GpSimd: h' = (h - zb*h) + zb*n
                            w = gp.tile([128, HALF], BF, tag="w0")
                            y = gp.tile([128, HALF], BF, tag="y0")
                            v = gp.tile([128, HALF], BF, tag="v0")
                            nc.vector.tensor_mul(v[:], zb[:], n[:])
                            nc.gpsimd.tensor_tensor(w[:], zb[:], h[:], MULT)
                            nc.gpsimd.tensor_tensor(y[:], h[:], w[:], mybir.AluOpType.subtract)
                            nc.vector.tensor_add(hn2[:], y[:], v[:])
                        else:
                            d = gp.tile([128, HALF], BF, tag=f"d{s}")
                            e = gp.tile([128, HALF], BF, tag=f"e{s}")
                            nc.vector.tensor_sub(d[:], n[:], h[:])
                            nc.vector.tensor_mul(e[:], zb[:], d[:])
                            nc.vector.tensor_add(hn2[:], h[:], e[:])
                        h_cur[s] = hn2

            ps_l = psp.tile([A, BL], F32, tag="logits")
            nc.tensor.matmul(ps_l[:, 0:HALF], wout_sb[:], h_cur[0][:], start=True, stop=True)
            nc.tensor.matmul(ps_l[:, HALF:BL], wout_sb[:], h_cur[1][:], start=True, stop=True)
            expv = cp.tile([A, BL], F32, tag="expv")
            nc.scalar.activation(expv[:], ps_l[:], EXP, bias=bout_sb[:, 0:1])
            nc.sync.dma_start(out_d[:], expv[:])

    nc.finalize()
    return nc


def _prep_host(utterance, emb_table, w_ih, w_hh, b_ih, b_hh, w_out, b_out):
    utt = np.asarray(utterance).astype(np.int64)
    emb = np.asarray(emb_table).astype(np.float32)
    w_ih = np.asarray(w_ih).astype(np.float32)
    w_hh = np.asarray(w_hh).astype(np.float32)
    b_ih = np.asarray(b_ih).astype(np.float32)
    b_hh = np.asarray(b_hh).astype(np.float32)
    w_out = np.asarray(w_out).astype(np.float32)
    b_out = np.asarray(b_out).astype(np.float32)

    # --- death-step index rewrite (sentinel = row V1) ---
    nz = utt != 0                                  # [B, T]
    alive0 = np.ones((B, 1), bool)
    alive_t = np.concatenate([alive0, np.cumprod(nz[:, :-1], axis=1).astype(bool)], axis=1)
    idx = np.where(alive_t, utt, V1).astype(np.int32)     # [B, T]

    # --- pre-projected gate table [i_r+b_r | -(i_z+b_z) | i_n+b_ihn] bf16 ---
    b_r = b_ih[0:E] + b_hh[0:E]
    b_z = b_ih[E:2 * E] + b_hh[E:2 * E]
    table = np.zeros((V1 + 1, 3, E), BF16)
    table[:V1, 0] = (emb @ w_ih[0:E].T + b_r).astype(BF16)
    table[:V1, 1] = (-(emb @ w_ih[E:2 * E].T) - b_z).astype(BF16)
    table[:V1, 2] = (emb @ w_ih[2 * E:3 * E].T + b_ih[2 * E:3 * E]).astype(BF16)
    table[V1, 1] = np.float32(-30000.0)            # dead: zbar = sigmoid(-30000+h_z) = 0
    table_u16 = table.view(np.uint16)              # [V1+1, 3, E]

    # --- dense per-core projection stream [NG, 128, GS*3*BL] bf16 ---
    streams = []
    for cix in range(NCORES):
        ids = idx[cix * BL:(cix + 1) * BL]         # [BL, T]
        gat = table_u16[ids]                       # [BL, T, 3, E] u16
        gat = gat.reshape(BL, NG, GS, 3, E)
        st = np.ascontiguousarray(np.transpose(gat, (1, 4, 2, 3, 0)))  # [NG, E, GS, 3, BL]
        streams.append(st.reshape(NG, 128, GS * 3 * BL).view(BF16))

    ident = np.eye(E, dtype=np.float32)
    wstat = np.concatenate(
        [ident, w_hh[0:E].T, -w_hh[E:2 * E].T, w_hh[2 * E:3 * E].T], axis=1
    ).astype(BF16)                                  # [128, 512]
    biasp = b_hh[2 * E:3 * E].reshape(E, 1).astype(np.float32)
    woutT = np.ascontiguousarray(w_out.T).astype(BF16)   # [128, 10]
    bout = b_out.reshape(A, 1).astype(np.float32)

    shared = {"wstat": wstat, "biasp": biasp, "woutT": woutT, "bout": bout}
    return [dict(shared, estream=streams[c]) for c in range(NCORES)]


def kernel(utterance, global_idxes, emb_table, w_ih, w_hh, b_ih, b_hh, w_out, b_out):
    in_maps = _prep_host(utterance, emb_table, w_ih, w_hh, b_ih, b_hh, w_out, b_out)
    if "nc" not in _CACHE:
        _CACHE["nc"] = _build_nc()
    nc = _CACHE["nc"]
    res = run_bass_kernel_spmd(nc, in_maps, core_ids=list(range(NCORES)))
    out = np.empty((B, A), np.float64)
    for c in range(NCORES):
        expv = res.results[c]["expv"].astype(np.float64)       # [A, BL]
        out[c * BL:(c + 1) * BL] = (expv / expv.sum(axis=0, keepdims=True)).T
    return out.astype(np.float32)


# revision 8
# speedup vs baseline: 1.1571x; 1.1571x over previous
"""Trainium2 Bass kernel for nn_AgentTwo (ragged-sequence GRU agent).

Full-input contract: kernel(**inputs) takes the unsharded numpy inputs and
returns the full [8192, 10] float32 action probabilities.

Strategy (pure data parallel over 8 NeuronCores, B=8192 -> 1024 rows/core):
 - Host resolves the ragged aliveness up front: per row, tokens at steps at
   or after the first zero are rewritten to a sentinel whose (pre-negated)
   z-gate projection is -30000, so sigmoid saturates to 0 and h freezes
   exactly on device -- the reference's "output_state while alive" semantics
   fall out with zero extra device work.
 - Host resolves the embedding lookup AND the input-side projections: the
   per-core bf16 stream carries, per token, [i_r+b_r | -(i_z)-b_z | i_n+b_ihn]
   in [E, B] layout (E on partitions). The device then:
     * loads the projections into PSUM via an identity matmul and
       accumulates the hidden-side matmuls on top (PE, bf16 in / f32 acc),
     * applies BOTH sigmoids in a single unbiased ACTIVATE over the
       adjacent r/zneg PSUM banks,
     * runs the n-gate and the frozen-blend on DVE:
         tg   = (psum_hn + b_hhn) * r           (scalar_tensor_tensor)
         npre = tg + gi_n ; n = tanh(npre)      (DVE + ACT)
         h'   = h + zbar * (n - h)              (DVE x3, bf16)
   The batch runs as two independent 512-column streams so the two
   recurrence dependency chains pipeline across engines.
 - Head: logitsT = w_out @ h (PE), expv = exp(logitsT + b_out) (ACT); host
   normalizes the softmax in f64 and reassembles [8192, 10].
"""

import sys

for _p in ("/opt/trn_rl_repo",):
    if _p not in sys.path:
        sys.path.append(_p)

import numpy as np
import ml_dtypes

import concourse.bass as bass
import concourse.mybir as mybir
import concourse.tile as tile
from concourse import bacc
from concourse.bass_utils import run_bass_kernel_spmd

BF16 = ml_dtypes.bfloat16

NCORES = 8
B, T, E, V, A = 8192, 64, 128, 32000, 10
V1 = V + 1          # vocab rows (0..32000)
BL = B // NCORES    # 1024 rows per core
HALF = BL // 2      # 512-column stream width
GS = 2              # timesteps per stream DMA
NG = T // GS        # stream groups
F32 = mybir.dt.float32
BF = mybir.dt.bfloat16

_CACHE = {}


def _build_nc(T=T, BL=BL, NG=NG):
    HALF = BL // 2
    nc = bacc.Bacc(None)
    es_d = nc.declare_dram_parameter("estream", [NG, 128, GS * 3 * BL], BF, isOutput=False)
    w_d = nc.declare_dram_parameter("wstat", [128, 4 * E], BF, isOutput=False)
    bias_d = nc.declare_dram_parameter("biasp", [128, 1], F32, isOutput=False)
    wout_d = nc.declare_dram_parameter("woutT", [128, A], BF, isOutput=False)
    bout_d = nc.declare_dram_parameter("bout", [A, 1], F32, isOutput=False)
    out_d = nc.declare_dram_parameter("expv", [A, BL], F32, isOutput=True)

    SIG = mybir.ActivationFunctionType.Sigmoid
    TANH = mybir.ActivationFunctionType.Tanh
    EXP = mybir.ActivationFunctionType.Exp
    ADD = mybir.AluOpType.add
    MULT = mybir.AluOpType.mult

    with tile.TileContext(nc) as tc:
        with (
            tc.tile_pool(name="const", bufs=1) as cp,
            tc.tile_pool(name="gath", bufs=4) as gathp,
            tc.tile_pool(name="hA", bufs=3) as hpA,
            tc.tile_pool(name="hB", bufs=3) as hpB,
            tc.tile_pool(name="gates", bufs=3) as gp,
            tc.tile_pool(name="ps", bufs=1, space=bass.MemorySpace.PSUM) as psp,
        ):
            w_sb = cp.tile([128, 4 * E], BF, tag="w")
            bias_sb = cp.tile([128, 1], F32, tag="bias")
            wout_sb = cp.tile([128, A], BF, tag="wout")
            bout_sb = cp.tile([A, 1], F32, tag="bout")
            nc.sync.dma_start(w_sb[:], w_d[:])
            nc.sync.dma_start(bias_sb[:], bias_d[:])
            nc.sync.dma_start(wout_sb[:], wout_d[:])
            nc.sync.dma_start(bout_sb[:], bout_d[:])

            # weight column slices in w_sb: [I | hhr | -hhz | hhn]
            W_ID = w_sb[:, 0 * E:1 * E]
            W_HHR = w_sb[:, 1 * E:2 * E]
            W_HHZN = w_sb[:, 2 * E:3 * E]
            W_HHN = w_sb[:, 3 * E:4 * E]
            B_HHN = bias_sb[:, 0:1]

            h_cur = []
            for s, hp in ((0, hpA), (1, hpB)):
                h0 = hp.tile([128, HALF], BF, tag=f"h{s}")
                nc.vector.memset(h0[:], 0.0)
                h_cur.append(h0)

            for g in range(NG):
                ep = gathp.tile([128, GS, 3, BL], BF, tag="ep")
                nc.sync.dma_start(ep[:], es_d[g])
                for k in range(GS):
                    for s in range(2):
                        lo = s * HALF
                        hi = lo + HALF
                        prT = ep[:, k, 0, lo:hi]
                        pzT = ep[:, k, 1, lo:hi]
                        pnT = ep[:, k, 2, lo:hi]
                        h = h_cur[s]

                        ps_rz = psp.tile([128, 2 * HALF], F32, tag=f"rz{s}")
                        ps_hn = psp.tile([128, HALF], F32, tag=f"hn{s}")
                        # projection loads first: they only need the stream,
                        # so PE can run them while waiting for h'
                        nc.tensor.matmul(ps_rz[:, 0:HALF], W_ID, prT, start=True, stop=False)
                        nc.tensor.matmul(ps_rz[:, HALF:], W_ID, pzT, start=True, stop=False)
                        nc.tensor.matmul(ps_rz[:, 0:HALF], W_HHR, h[:], start=False, stop=True)
                        nc.tensor.matmul(ps_hn[:], W_HHN, h[:], start=True, stop=True)
                        nc.tensor.matmul(ps_rz[:, HALF:], W_HHZN, h[:], start=False, stop=True)

                        r = gp.tile([128, HALF], BF, tag=f"r{s}")
                        zb = gp.tile([128, HALF], BF, tag=f"zb{s}")
                        tg = gp.tile([128, HALF], BF, tag=f"tg{s}")
                        npre = gp.tile([128, HALF], BF, tag=f"np{s}")
                        n = gp.tile([128, HALF], BF, tag=f"n{s}")
                        hn2 = (hpA if s == 0 else hpB).tile([128, HALF], BF, tag=f"h{s}")

                        d = gp.tile([128, HALF], BF, tag=f"d{s}")
                        e = gp.tile([128, HALF], BF, tag=f"e{s}")
                        nc.scalar.activation(r[:], ps_rz[:, 0:HALF], SIG)
                        nc.scalar.activation(zb[:], ps_rz[:, HALF:], SIG)
                        nc.vector.scalar_tensor_tensor(tg[:], ps_hn[:], B_HHN, r[:], ADD, MULT)
                        nc.vector.tensor_add(npre[:], tg[:], pnT)
                        nc.scalar.activation(n[:], npre[:], TANH)
                        nc.vector.tensor_sub(d[:], n[:], h[:])
                        nc.vector.tensor_mul(e[:], zb[:], d[:])
                        nc.vector.tensor_add(hn2[:], h[:], e[:])
                        h_cur[s] = hn2

            ps_l = psp.tile([A, BL], F32, tag="logits")
            nc.tensor.matmul(ps_l[:, 0:HALF], wout_sb[:], h_cur[0][:], start=True, stop=True)
            nc.tensor.matmul(ps_l[:, HALF:BL], wout_sb[:], h_cur[1][:], start=True, stop=True)
            expv = cp.tile([A, BL], F32, tag="expv")
            nc.scalar.activation(expv[:], ps_l[:], EXP, bias=bout_sb[:, 0:1])
            nc.sync.dma_start(out_d[:], expv[:])

    nc.finalize()
    return nc


def _prep_host(utterance, emb_table, w_ih, w_hh, b_ih, b_hh, w_out, b_out):
    utt = np.asarray(utterance).astype(np.int64)
    emb = np.asarray(emb_table).astype(np.float32)
    w_ih = np.asarray(w_ih).astype(np.float32)
    w_hh = np.asarray(w_hh).astype(np.float32)
    b_ih = np.asarray(b_ih).astype(np.float32)
    b_hh = np.asarray(b_hh).astype(np.float32)
    w_out = np.asarray(w_out).astype(np.float32)
    b_out = np.asarray(b_out).astype(np.float32)

    # --- death-step index rewrite (sentinel = row V1) ---
    nz = utt != 0                                  # [B, T]
    alive0 = np.ones((B, 1), bool)
    alive_t = np.concatenate([alive0, np.cumprod(nz[:, :-1], axis=1).astype(bool)], axis=1)
    idx = np.where(alive_t, utt, V1).astype(np.int32)     # [B, T]

    # --- pre-projected gate table [i_r+b_r | -(i_z+b_z) | i_n+b_ihn] bf16 ---
    b_r = b_ih[0:E] + b_hh[0:E]
    b_z = b_ih[E:2 * E] + b_hh[E:2 * E]
    table = np.zeros((V1 + 1, 3, E), BF16)
    table[:V1, 0] = (emb @ w_ih[0:E].T + b_r).astype(BF16)
    table[:V1, 1] = (-(emb @ w_ih[E:2 * E].T) - b_z).astype(BF16)
    table[:V1, 2] = (emb @ w_ih[2 * E:3 * E].T + b_ih[2 * E:3 * E]).astype(BF16)
    table[V1, 1] = np.float32(-30000.0)            # dead: zbar = sigmoid(-30000+h_z) = 0
    table_u16 = table.view(np.uint16)              # [V1+1, 3, E]

    # --- dense per-core projection stream [NG, 128, GS*3*BL] bf16 ---
    streams = []
    for cix in range(NCORES):
        ids = idx[cix * BL:(cix + 1) * BL]         # [BL, T]
        gat = table_u16[ids]                       # [BL, T, 3, E] u16
        gat = gat.reshape(BL, NG, GS, 3, E)
        st = np.ascontiguousarray(np.transpose(gat, (1, 4, 2, 3, 0)))  # [NG, E, GS, 3, BL]
        streams.append(st.reshape(NG, 128, GS * 3 * BL).view(BF16))

    ident = np.eye(E, dtype=np.float32)
    wstat = np.concatenate(
        [ident, w_hh[0:E].T, -w_hh[E:2 * E].T, w_hh[2 * E:3 * E].T], axis=1
    ).astype(BF16)                                  # [128, 512]
    biasp = b_hh[2 * E:3 * E].reshape(E, 1).astype(np.float32)
    woutT = np.ascontiguousarray(w_out.T).astype(BF16)   # [128, 10]
    bout = b_out.reshape(A, 1).astype(np.float32)

    shared = {"wstat": wstat, "biasp": biasp, "woutT": woutT, "bout": bout}
    return [dict(shared, estream=streams[c]) for c in range(NCORES)]


def kernel(utterance, global_idxes, emb_table, w_ih, w_hh, b_ih, b_hh, w_out, b_out):
    in_maps = _prep_host(utterance, emb_table, w_ih, w_hh, b_ih, b_hh, w_out, b_out)
    if "nc" not in _CACHE:
        _CACHE["nc"] = _build_nc()
    nc = _CACHE["nc"]
    res = run_bass_kernel_spmd(nc, in_maps, core_ids=list(range(NCORES)))
    out = np.empty((B, A), np.float64)
    for c in range(NCORES):
        expv = res.results[c]["expv"].astype(np.float64)       # [A, BL]
        out[c * BL:(c + 1) * BL] = (expv / expv.sum(axis=0, keepdims=True)).T
    return out.astype(np.float32)
